# revision 51
# baseline (speedup 1.0000x reference)
"""Viterbi decode (CRF) kernel for Trainium2, data-parallel over batch on 8 cores.

Split-direction Viterbi: a forward max-plus pass over positions 0..M and a
backward pass over S-1..M run concurrently (two independent carry chains), then
the best meeting tag at M is picked and TWO independent backtrace chains walk
outward (M->0 and M->S-1), interleaved to hide each other's latency.

The per-step broadcast-add + segmented max-reduce is fused into ONE custom DVE
instruction (ADD_SEGMAX): a MAX-scan over (carry[b,q] + trans[q,c]) whose
running value is re-seeded at each 32-element page boundary via a hand-built
SUB_DIM_DONE step state; the per-page max is read back from the last element
of each page. The feat-add runs on GPSIMD, off the DVE critical path.

The backtrace recomputes the argmax only along the traced path: the needed
trans column/row is selected by 4 concurrent 32x32 PE matmuls on a one-hot,
then a fused add+max custom DVE op (ADD_MAXREDUCE) + MaxIndex give the tag.
"""

import dataclasses as _dc
import sys

sys.path.insert(0, "/opt/trn_rl_repo")

import numpy as np

from concourse import bass, mybir
from concourse import dve_ops as _dve_ops
from concourse.dve_ops import DveOp, _COMPILE_CACHE
from concourse.dve_spec import (
    AluOp as _AluOp,
    Latch as _Latch,
    MaxNeg as _MaxNeg,
    N_LANES as _N_LANES,
    N_STAGES as _N_STAGES,
    Scan as _Scan,
    Spec as _Spec,
    Src0 as _Src0,
    Src1 as _Src1,
    Trigger as _Trigger,
    _assemble,
    _build_placement,
    _build_state_machine,
    _collect,
    _hoist_stream_invariant_ops,
    _Stage,
    _validate_body,
    lower as _lower,
    scan as _scan,
)
from concourse.dve_uop import DveOpSpec
from concourse.tile import TileContext

F32 = mybir.dt.float32
I32 = mybir.dt.int32
U32 = mybir.dt.uint32

B_LOC = 128  # batch rows per core
T = 32  # tags
N_CORES = 8

# ---------------------------------------------------------------------------
# Custom DVE ops
# ---------------------------------------------------------------------------


def _segmax_ref(in0, in1, c0, c1, c2):
    x = in0.astype(np.float32) + in1.astype(np.float32)
    return np.maximum.accumulate(x, axis=-1)


def _addmax_ref(in0, in1, c0, c1, c2):
    x = in0.astype(np.float32) + in1.astype(np.float32)
    acc = x.reshape(x.shape[0], -1).max(axis=-1, keepdims=True)
    return x, acc


def _lower_segmax(spec, ver):
    """Stock [seed, steady] machine plus a SUB_DIM_DONE step state that
    re-seeds the MAX-scan (max(-FLT_MAX, expr) = expr) while consuming the
    first element of each page -> per-page segmented running max."""
    n_lanes, n_stages = _N_LANES[ver], _N_STAGES[ver]
    _validate_body(spec, ver)
    spec = _hoist_stream_invariant_ops(spec)
    scans = _collect(spec.body, _Scan)
    latches = _collect(spec.body, _Latch)
    assert len(scans) == 1 and not latches
    p = _build_placement(spec, scans, n_stages, n_lanes)
    states = _build_state_machine(spec, scans, latches, p)
    assert len(states) == 2, f"expected [seed, steady], got {len(states)}"
    seed, steady = states
    sc = scans[0]
    d = p.node_stage[sc]
    steady2 = _dc.replace(
        steady,
        trigger=(_Trigger.SRC_TENSOR_DONE, _Trigger.SUB_DIM_DONE, _Trigger.NONE),
        next=(0, 2, 0),
    )
    step = _dc.replace(
        steady,
        overrides={**steady.overrides, d: _Stage(_AluOp.MAX, _MaxNeg, sc.expr)},
        trigger=(_Trigger.SRC_TENSOR_DONE, _Trigger.SUB_DIM_DONE, _Trigger.COUNT),
        next=(0, 2, 1),
        repeat=1,
    )
    out = [_assemble(s) for s in (seed, steady2, step)]
    for u in out:
        u.validate(ver)
    return out


@_dc.dataclass(frozen=True)
class _HandOp(DveOp):
    """DveOp compiled by a custom lowering; sha pin skipped (computed live)."""

    def compile(self, ver):
        key = (self.name, ver)
        if (r := _COMPILE_CACHE.get(key)) is not None:
            return r
        uops = (
            _lower_segmax(self.spec, ver)
            if self.name == "ANT_ADD_SEGMAX"
            else _lower(self.spec, ver=ver)
        )
        result = DveOpSpec(
            name=self.name,
            opcode=_dve_ops.get_dve_sub_opcode(self.name),
            uops=uops,
            rd1_en=True,
        )
        _COMPILE_CACHE[key] = result
        return result


ADD_SEGMAX = _HandOp(
    "ANT_ADD_SEGMAX",
    _Spec(body=_scan(_AluOp.MAX, _Src0 + _Src1, init=_MaxNeg), reference=_segmax_ref),
    subdim=True,
    uops_sha={},
)

ADD_MAXREDUCE = _HandOp(
    "ANT_ADD_MAXREDUCE",
    _Spec(body=_Src0 + _Src1, accum=_AluOp.MAX, reference=_addmax_ref),
    subdim=False,
    uops_sha={},
)


def _register(op):
    if op.name in _dve_ops._SUB_OPCODE_FOR_NAME:
        return
    _dve_ops.OPS.append(op)
    _dve_ops._SUB_OPCODE_FOR_NAME[op.name] = (
        _dve_ops._CUSTOM_DVE_ROW_BASE + len(_dve_ops.OPS) - 1
    )
    _dve_ops.CUSTOM_DVE_SPECS[op.name] = op.spec
    assert max(_dve_ops._SUB_OPCODE_FOR_NAME.values()) < 0x20


_register(ADD_SEGMAX)
_register(ADD_MAXREDUCE)

# ---------------------------------------------------------------------------
# Kernel build
# ---------------------------------------------------------------------------

NCONST = 2 * T * T + 7 * T  # transT, transB, transmmF, transmmB, start, stop, idxW, iota2


def build_nc(S: int, fix_waits: bool = True):
    M = S // 2
    nc = bass.Bass()

    feats_d = nc.declare_dram_parameter("feats", [B_LOC, S, T], F32, isOutput=False)
    consts_d = nc.declare_dram_parameter("consts", [B_LOC, NCONST], F32, isOutput=False)
    # path layout: [32, S, 4] flattened; path[32*k + b', s] = dram[b', s*4+k].
    # Kept fp32 end-to-end (host converts): HW tensor_copy fp32->int32
    # bitcasts rather than converts.
    path_d = nc.declare_dram_parameter("path", [32, S * 4], F32, isOutput=True)

    add = mybir.AluOpType.add
    iseq = mybir.AluOpType.is_equal

    with TileContext(nc) as tc:
        with (
            tc.tile_pool(name="const", bufs=1) as cpool,
            tc.tile_pool(name="featp", bufs=1) as fpool,
            tc.tile_pool(name="work", bufs=1) as wpool,
            tc.tile_pool(name="scr", bufs=2) as spool,
            tc.tile_pool(name="psum", bufs=2, space="PSUM") as ppool,
            tc.tile_pool(name="psumIdx", bufs=1, space="PSUM") as ipool,
        ):
            consts_t = cpool.tile([B_LOC, NCONST], F32)
            nc.sync.dma_start(out=consts_t[:], in_=consts_d[:])
            o = 0
            transT3 = consts_t[:, o : o + T * T].rearrange("p (c q) -> p c q", q=T)
            o += T * T
            transB3 = consts_t[:, o : o + T * T].rearrange("p (m r) -> p m r", r=T)
            o += T * T
            transmmF = consts_t[:, o : o + T]
            o += T
            transmmB = consts_t[:, o : o + T]
            o += T
            start_t = consts_t[:, o : o + T]
            o += T
            stop_t = consts_t[:, o : o + T]
            o += T
            # idxW[p, k] = p%32 if p//32==k else 0: one PE matmul with lhsT =
            # the block-diagonal one-hot ohT gives FpIdx[b', k] = argmax tag
            # of batch row 32k+b' (the path value), on 32 partitions x 4.
            idxW = consts_t[:, o : o + 4]
            o += T
            iota2_t = consts_t[:, o : o + 2 * T]

            # ft arrives in interleaved lo/hi chunks on the ACT engine's DMA
            # queue (serial per queue, concurrent with the consts DMA on SP's)
            # so both passes can start ~20us before the full tensor lands.
            ft = fpool.tile([B_LOC, S, T], F32)
            NCH = 4
            ck = S // (2 * NCH) if S >= 2 * NCH else S
            if ck < S:
                for k in range(NCH):
                    lo = k * ck
                    nc.scalar.dma_start(
                        out=ft[:, lo : lo + ck, :], in_=feats_d[:, lo : lo + ck, :]
                    )
                    hi = S - (k + 1) * ck
                    nc.scalar.dma_start(
                        out=ft[:, hi : hi + ck, :], in_=feats_d[:, hi : hi + ck, :]
                    )
            else:
                nc.scalar.dma_start(out=ft[:], in_=feats_d[:])

            carF = wpool.tile([B_LOC, M + 1, T], F32)
            vpB = wpool.tile([B_LOC, M, T], F32)  # vpB[:, j] = vplus_{M+1+j}
            path_t = wpool.tile([32, S * 4], F32)

            # one DMA-wait touch per (engine, DMA) pair; later consumers
            # inherit via engine program order.
            tt0 = wpool.tile([B_LOC, 1], F32, tag="touch0")
            nc.vector.tensor_copy(tt0[:], consts_t[:, 0:1])
            tt1 = wpool.tile([B_LOC, 1], F32, tag="touch1")
            nc.vector.tensor_copy(tt1[:], ft[:, 0, 0:1])
            tt2 = wpool.tile([B_LOC, 1], F32, tag="touch2")
            nc.gpsimd.tensor_copy(tt2[:], consts_t[:, 0:1])
            tt3 = wpool.tile([B_LOC, 1], F32, tag="touch3")
            nc.gpsimd.tensor_copy(tt3[:], ft[:, 0, 0:1])
            tt4 = wpool.tile([B_LOC, 1], F32, tag="touch4")
            nc.gpsimd.tensor_copy(tt4[:], ft[:, S - 1, 0:1])
            Fp0 = ppool.tile([B_LOC, T], F32, tag="FpF")
            nc.tensor.matmul(
                Fp0[0:32, :],
                transmmF[0:32, :],
                transmmF[0:32, :],
                start=True,
                stop=True,
                tile_position=(0, 0),
            )

            # ---------------- forward + backward passes ----------------
            nc.vector.tensor_tensor(carF[:, 0, :], ft[:, 0, :], start_t, op=add)

            def seg_add_max(scr, trans3, vec):
                vb = vec.unsqueeze(1).broadcast_to([B_LOC, T, T])
                nc.vector._custom_dve(ADD_SEGMAX, out=scr[:], in0=trans3, in1=vb)

            # round r: segB_r consumes vplus_{S-r} -> b_{S-1-r};
            #          segF_r consumes carF[r-1] -> carF[r]
            prevB = None  # scratchB holding b_{S-r} page-maxes at [:, :, T-1]
            for r in range(1, M + 1):
                if ck < S and r > 1 and (r - 1) % ck == 0:
                    # next ft lo/hi chunk pair: one Pool-side DMA-wait touch each
                    k = (r - 1) // ck
                    ttl = wpool.tile([B_LOC, 1], F32, tag=f"tchlo{k}")
                    nc.gpsimd.tensor_copy(ttl[:], ft[:, k * ck, 0:1])
                    tth = wpool.tile([B_LOC, 1], F32, tag=f"tchhi{k}")
                    nc.gpsimd.tensor_copy(tth[:], ft[:, S - (k + 1) * ck, 0:1])
                if r <= M - 1:
                    # featB: vplus_{S-r} = b_{S-r} + ft[S-r]
                    j = S - r - (M + 1)  # vpB slot for position S-r
                    bprev = stop_t if prevB is None else prevB[:, :, T - 1]
                    nc.gpsimd.tensor_tensor(vpB[:, j, :], bprev, ft[:, S - r, :], op=add)
                    scrB = spool.tile([B_LOC, T, T], F32, tag="scrB")
                    seg_add_max(scrB, transB3, vpB[:, j, :])
                    prevB = scrB
                scrF = spool.tile([B_LOC, T, T], F32, tag="scrF")
                seg_add_max(scrF, transT3, carF[:, r - 1, :])
                nc.gpsimd.tensor_tensor(
                    carF[:, r, :], scrF[:, :, T - 1], ft[:, r, :], op=add
                )

            # ---------------- stitch ----------------
            # One-hot of the meeting tag comes straight from is_equal(tot, max)
            # (no MaxIndex): fp32 sums of distinct random values tie with
            # negligible probability, and the reduce's accumulator sees the
            # exact same fp32 values it writes, so eq(tot, mx) is exact.
            tot = wpool.tile([B_LOC, T], F32)
            mxS = wpool.tile([B_LOC, 1], F32)
            nc.vector._custom_dve(
                ADD_MAXREDUCE,
                out=tot[:],
                in0=carF[:, M, :],
                in1=prevB[:, :, T - 1],
                accum_out=mxS[:],
            )

            # ---------------- backtrace (two independent chains) ----------------
            # Per chain step (all-DVE serial segment, then one PE hop):
            #   ADD_MAXREDUCE(tot = scores + gathered trans column, mx)
            #   is_equal(oh = tot == mx)           <- next step's one-hot
            #   transpose(ohT, 32x32 blocks)
            #   4x PE matmul: moving operand carries trans AND an iota column,
            #     so Fp[:, T] is the argmax INDEX (the path tag) directly.
            #   ACT copies Fp[:, T] into the int32 path tile (cast), off the
            #   critical chain.
            # One-hot buffers are 2T wide with the upper half zeroed once:
            # StreamTranspose on a [128, 32] fp32 operand returns garbage on
            # hardware (verified empirically); the baseline's working shape
            # was [128, 64]. eq writes cols 0:T, the transpose covers 2T, PE
            # reads cols 0:T of the transposed result.
            #
            # Deliberate aliases: the B chain's maxreduce output (totB) reuses
            # bufF (the F one-hot) and the F chain's (totF) reuses bufB. The
            # WAR hazards (each chain's maxreduce overwrites what the other
            # chain's transpose just consumed) pin the scheduler to strict
            # round-robin F-block/B-block DVE order, so each chain's serial
            # DVE segment sits inside the other chain's PE round-trip window.
            # Dataflow stays correct: every aliased value is consumed before
            # its buffer is overwritten (MR_x -> eq_x -> TR_x within a block).
            bufF = wpool.tile([B_LOC, 2 * T], F32)
            bufB = wpool.tile([B_LOC, 2 * T], F32)
            ohTF = wpool.tile([B_LOC, 2 * T], F32)
            ohTB = wpool.tile([B_LOC, 2 * T], F32)
            nc.gpsimd.memset(bufF[:, T:], 0.0)
            nc.gpsimd.memset(bufB[:, T:], 0.0)
            ohF = bufF[:, 0:T]
            totB = bufF[:, 0:T]
            totF = bufB[:, 0:T]
            ohB = bufB[:, 0:T]
            mxF = wpool.tile([B_LOC, 1], F32)
            mxB = wpool.tile([B_LOC, 1], F32)

            # ACT touches every ft chunk + consts once (cheap, long before
            # phase 2 needs them): the final path DMA's queue-order wait then
            # becomes transitively implied by its ACT-completion wait and is
            # stripped. Allocated after the backtrace work tiles so pool-slot
            # reuse deps don't attach ACT waits to backtrace instructions.
            if ck < S:
                for k in range(NCH):
                    ttA = wpool.tile([B_LOC, 1], F32, tag=f"touchA{k}")
                    nc.scalar.copy(ttA[:], ft[:, k * ck, 0:1])
                    ttB = wpool.tile([B_LOC, 1], F32, tag=f"touchB{k}")
                    nc.scalar.copy(ttB[:], ft[:, S - (k + 1) * ck, 0:1])
            else:
                ttA = wpool.tile([B_LOC, 1], F32, tag="touchA")
                nc.scalar.copy(ttA[:], ft[:, 0, 0:1])
            ttC = wpool.tile([B_LOC, 1], F32, tag="touchC")
            nc.scalar.copy(ttC[:], consts_t[:, 0:1])

            def oh_transpose(oh, buf, ohT, tot_t, mx_t):
                # tensor_scalar with a per-partition [p,1] scalar operand: a
                # stride-0 broadcast on tensor_tensor silently reads garbage
                # on hardware (CoreSim models it fine — verified empirically).
                nc.vector.tensor_scalar(oh, tot_t, mx_t[:, 0:1], None, op0=iseq)
                nc.vector.transpose(ohT[:], buf[:])

            # path values accumulate directly in a persistent PSUM buffer
            # [32, S, 4] (block-transposed batch layout, see idxW): the idx
            # matmul of each backtrace step writes its 4-float result at its
            # position — no per-step copy-out instruction, no WAR hazards
            # (each position is written once), nothing but PE touches it until
            # the final copy. Host code undoes the layout.
            pathP = ipool.tile([32, S, 4], F32)

            def pe_select(ohT, transmm, tag, pos):
                # idx matmul FIRST so the chain's maxreduce (which waits on
                # the last select) transitively covers every reader of ohT —
                # the wait-strip pass needs that to resolve the transpose's
                # WAR wait. pos=None skips it (stitch emits the meeting tag
                # once, via the F-side call).
                if pos is not None:
                    nc.tensor.matmul(
                        pathP[:, pos, :],
                        ohT[:, 0:T],
                        idxW,
                        start=True,
                        stop=True,
                        tile_position=(0, 0),
                    )
                Fp = ppool.tile([B_LOC, T], F32, tag=tag)
                for k in range(4):
                    nc.tensor.matmul(
                        Fp[32 * k : 32 * k + 32, :],
                        ohT[32 * k : 32 * k + 32, 0:T],
                        transmm[32 * k : 32 * k + 32, :],
                        start=True,
                        stop=True,
                        tile_position=(32 * k, 32 * k),
                    )
                return Fp

            def add_max(scores, Fp, tot_t, mx_t):
                nc.vector._custom_dve(
                    ADD_MAXREDUCE,
                    out=tot_t[:],
                    in0=scores,
                    in1=Fp[:],
                    accum_out=mx_t[:],
                )

            # both chains start from the same meeting one-hot. FpB is emitted
            # first so the F chain's first maxreduce waits on the HIGHEST PE
            # sem value, which lets the wait-strip pass resolve the round-0
            # transpose's PE WAR wait by transitivity.
            oh_transpose(ohF, bufF, ohTF, tot[:], mxS)
            FpB = pe_select(ohTF, transmmB, "FpB", None)
            FpF = pe_select(ohTF, transmmF, "FpF", M)

            # F chain: step t (t=0..M-1) consumes FpF for position i=M-t and
            # produces path[M-t-1]. B chain: step t (t=0..M-2) consumes FpB
            # for position i=M+t and produces path[M+t+1].
            for t in range(M):
                add_max(carF[:, M - t - 1, :], FpF, totF, mxF)
                oh_transpose(ohF, bufF, ohTF, totF, mxF)
                FpF = pe_select(ohTF, transmmF, "FpF", M - t - 1)
                if t <= M - 2:
                    add_max(vpB[:, t, :], FpB, totB, mxB)
                    oh_transpose(ohB, bufB, ohTB, totB, mxB)
                    FpB = pe_select(ohTB, transmmB, "FpB", M + t + 1)

            nc.vector.tensor_copy(path_t[:], pathP[:].rearrange("p a b -> p (a b)"))
            nc.sync.dma_start(out=path_d[:], in_=path_t[:])

    mybir.codegen_inst_isa_subclasses(nc)
    if fix_waits:
        _strip_redundant_pe_waits(nc)
    return nc


def _strip_redundant_pe_waits(nc):
    """Walrus encodes at most one sync-wait per compute instruction.

    1. Merge multiple waits on the same semaphore to the max value.
    2. Split multi-wait drains into chains of single-wait drains.
    3. Drop a non-DVE wait on a DVE-waiting instruction when some DVE
       instruction with completion tick <= the DVE wait value already waited
       on that semaphore >= the required value (transitive implication that
       Tile doesn't minimize across procs)."""
    f = nc.m.functions[0]
    insts = [i for blk in f.blocks for i in blk.instructions]

    from concourse import mybir as _mybir
    import copy as _copy

    # 1. same-sem merge
    for inst in insts:
        si = inst.sync_info
        if si is None or not si.on_wait or len(si.on_wait) <= 1:
            continue
        best = {}
        for w in si.on_wait:
            k = w.ant_name
            if k not in best or w.wait_value > best[k].wait_value:
                best[k] = w
        if len(best) < len(si.on_wait):
            inst.sync_info = _mybir.SyncInfo(
                on_wait=list(best.values()), on_update=list(si.on_update or [])
            )



    # 1b. drop same-engine self-waits whose producer is at least THREE
    # instructions back in the engine's own queue: in-order issue plus the
    # intervening ops' execution time guarantees the producer's SBUF writes
    # are visible. Adjacent (or 2-back) RAW pairs KEEP their sem: hardware
    # makes an op's writes visible ~95ns after its pipe drain, so the next
    # op's reads can race them (observed empirically — a transpose reading
    # its is_equal predecessor's output returned stale data when stripped).
    own_count: dict = {}
    for inst in insts:
        si = inst.sync_info
        eng = str(inst.engine).split(".")[-1]
        if si is not None and si.on_wait:
            keep = [
                w
                for w in si.on_wait
                if not (
                    w.ant_name
                    and w.ant_name.split("_")[0] == eng
                    and w.wait_value <= own_count.get(w.ant_name, 0) - 2
                )
            ]
            if len(keep) < len(si.on_wait):
                inst.sync_info = _mybir.SyncInfo(
                    on_wait=keep, on_update=list(si.on_update or [])
                )
        si = inst.sync_info
        if si is not None:
            for u in si.on_update or []:
                if u.ant_name:
                    own_count[u.ant_name] = own_count.get(u.ant_name, 0) + u.update_value

    # cumulative: after the k-th E-sem increment, the largest value of each
    # other semaphore that engine E has (transitively) waited on so far.
    def prefix(name):
        return name.split("_")[0]

    tick = {}  # sem prefix -> increments so far
    cur_max = {}  # engine prefix -> {other sem prefix -> max waited value}
    observed = {}  # (engine prefix, other prefix) -> [(tick, maxval)]
    for inst in insts:
        si = inst.sync_info
        if si is None:
            continue
        eng = str(inst.engine).split(".")[-1]
        cm = cur_max.setdefault(eng, {})
        for w in si.on_wait or []:
            if w.ant_name:
                p = prefix(w.ant_name)
                if p != eng:
                    cm[p] = max(cm.get(p, 0), w.wait_value)
        for u in si.on_update or []:
            if u.ant_name:
                p = prefix(u.ant_name)
                tick[p] = tick.get(p, 0) + u.update_value
                if p == eng:
                    for q, v in cm.items():
                        observed.setdefault((p, q), []).append((tick[p], v))

    def implied(via_prefix, via_val, other_name, other_val):
        """True when "via-sem >= via_val" transitively implies
        "other-sem >= other_val": the via engine had waited on other-sem
        >= other_val by the time it made its via_val-th increment."""
        p = prefix(other_name)
        best = 0
        for k, v in observed.get((via_prefix, p), []):
            if k <= via_val:
                best = max(best, v)
        return best >= other_val

    # 2. split multi-wait drains
    for blk in f.blocks:
        new_list = []
        for inst in blk.instructions:
            si = inst.sync_info
            if (
                type(inst).__name__ == "InstDrain"
                and si is not None
                and si.on_wait
                and len(si.on_wait) > 1
            ):
                waits = list(si.on_wait)
                for k, w in enumerate(waits[:-1]):
                    clone = _copy.copy(inst)
                    clone.name = f"{inst.name}-w{k}"
                    clone.sync_info = _mybir.SyncInfo(on_wait=[w], on_update=[])
                    new_list.append(clone)
                inst.sync_info = _mybir.SyncInfo(
                    on_wait=[waits[-1]], on_update=list(si.on_update or [])
                )
            new_list.append(inst)
        blk.instructions[:] = new_list

    # 3. transitivity strip: for each wait, check whether one of the OTHER
    # waits on the instruction already implies it; drop implied waits.
    for inst in insts:
        si = inst.sync_info
        if si is None or not si.on_wait or len(si.on_wait) <= 1:
            continue
        waits = list(si.on_wait)
        keep = []
        for i, w in enumerate(waits):
            redundant = any(
                implied(prefix(v.ant_name), v.wait_value, w.ant_name, w.wait_value)
                for j, v in enumerate(waits)
                if j != i and (v in keep or j > i)
            )
            if not redundant:
                keep.append(w)
        if len(keep) < len(waits):
            inst.sync_info = _mybir.SyncInfo(
                on_wait=keep, on_update=list(si.on_update or [])
            )

    # 4. fallback: split residual multi-wait compute instructions by hoisting
    # all but one wait onto single-wait drains inserted just before them on
    # the same engine (a satisfied drain retires in a few ns).
    n_split = 0
    for blk in f.blocks:
        new_list = []
        for inst in blk.instructions:
            si = inst.sync_info
            if (
                si is not None
                and si.on_wait
                and len(si.on_wait) > 1
                and type(inst).__name__
                not in ("InstDrain", "InstEventSemaphore", "InstISA", "InstCall")
            ):
                waits = list(si.on_wait)
                for k, w in enumerate(waits[:-1]):
                    d = _mybir.InstDrain(
                        name=f"{inst.name}-sw{k}",
                        ins=[],
                        outs=[],
                        bass_is_fusable=False,
                    )
                    d.engine = inst.engine
                    d.sync_info = _mybir.SyncInfo(on_wait=[w], on_update=[])
                    new_list.append(d)
                    n_split += 1
                inst.sync_info = _mybir.SyncInfo(
                    on_wait=[waits[-1]], on_update=list(si.on_update or [])
                )
            new_list.append(inst)
        blk.instructions[:] = new_list


def _make_const_inputs(transitions, start_transitions, stop_transitions):
    transitions = np.asarray(transitions, dtype=np.float32)
    start = np.asarray(start_transitions, dtype=np.float32)
    stop = np.asarray(stop_transitions, dtype=np.float32)
    consts = np.zeros((B_LOC, NCONST), dtype=np.float32)
    o = 0
    consts[:, o : o + T * T] = transitions.T.reshape(1, T * T)  # [c*T+q] = trans[q,c]
    o += T * T
    consts[:, o : o + T * T] = transitions.reshape(1, T * T)  # [m*T+r] = trans[m,r]
    o += T * T
    consts[:, o : o + T] = np.tile(transitions.T, (4, 1))  # transmmF[p,f]=trans[f,p%32]
    o += T
    consts[:, o : o + T] = np.tile(transitions, (4, 1))  # transmmB[p,f]=trans[p%32,f]
    o += T
    consts[:, o : o + T] = start[None, :]
    o += T
    consts[:, o : o + T] = stop[None, :]
    o += T
    # idxW[p, k] = p%32 if p//32==k else 0 (block-selective iota for the
    # path-index PE matmul); rest of the slot unused.
    p = np.arange(4 * T)
    consts[:, o : o + 4] = ((p[:, None] // T) == np.arange(4)[None, :]) * (
        p[:, None] % T
    ).astype(np.float32)
    o += T
    consts[:, o : o + 2 * T] = np.tile(np.arange(T, dtype=np.float32), 2)[None, :]
    return {"consts": consts}


class Runner:
    """Compile once, keep inputs device-resident, execute repeatedly."""

    def __init__(self, nc, n_cores=N_CORES):
        import jax
        from jax.sharding import Mesh, PartitionSpec, NamedSharding
        from jax.experimental.shard_map import shard_map
        from concourse import bass2jax

        self.jax = jax
        bass2jax.install_neuronx_cc_hook()

        partition_name = (
            nc.partition_id_tensor.name if nc.partition_id_tensor else None
        )
        in_names, out_names, out_avals, zero_outs = [], [], [], []
        for alloc in nc.m.functions[0].allocations:
            if not isinstance(alloc, mybir.MemoryLocationSet):
                continue
            name = alloc.memorylocations[0].name
            if alloc.kind == "ExternalInput":
                if name != partition_name:
                    in_names.append(name)
            elif alloc.kind == "ExternalOutput":
                out_names.append(name)
                shape = tuple(alloc.tensor_shape)
                dtype = mybir.dt.np(alloc.dtype)
                out_avals.append(jax.core.ShapedArray(shape, dtype))
                zero_outs.append(np.zeros((n_cores * shape[0], *shape[1:]), dtype))
        n_params = len(in_names)
        all_names = in_names + out_names
        if partition_name is not None:
            all_names = all_names + [partition_name]

        def _body(*args):
            operands = list(args)
            if partition_name is not None:
                operands.append(bass2jax.partition_id_tensor())
            outs = bass2jax._bass_exec_p.bind(
                *operands,
                out_avals=tuple(out_avals),
                in_names=tuple(all_names),
                out_names=tuple(out_names),
                lowering_input_output_aliases=(),
                sim_require_finite=True,
                sim_require_nnan=True,
                nc=nc,
            )
            return tuple(outs)

        self._body = _body
        devices = jax.devices()[:n_cores]
        assert len(devices) == n_cores
        self.mesh = Mesh(np.asarray(devices), ("core",))
        in_specs = (PartitionSpec("core"),) * (n_params + len(out_names))
        out_specs = (PartitionSpec("core"),) * len(out_names)
        self.sharded = jax.jit(
            shard_map(
                _body,
                mesh=self.mesh,
                in_specs=in_specs,
                out_specs=out_specs,
                check_rep=False,
            ),
            donate_argnums=tuple(range(n_params, n_params + len(out_names))),
            keep_unused=True,
        )
        self.sharding = NamedSharding(self.mesh, PartitionSpec("core"))
        self.in_names = in_names
        self.out_names = out_names
        self.out_avals = out_avals
        self.zero_outs = zero_outs
        self.n_cores = n_cores
        self.dev_in = None

    def set_inputs(self, in_maps):
        concat = [
            np.concatenate([np.asarray(m[name]) for m in in_maps], axis=0)
            for name in self.in_names
        ]
        self.dev_in = [self.jax.device_put(a, self.sharding) for a in concat]

    def execute(self):
        outs = self.sharded(*self.dev_in, *[z.copy() for z in self.zero_outs])
        outs = self.jax.block_until_ready(outs)
        return {
            name: np.asarray(outs[i]).reshape(
                self.n_cores, *self.out_avals[i].shape
            )
            for i, name in enumerate(self.out_names)
        }

    def make_chained(self, n_chain):
        """Callable dispatching the NEFF n_chain times, each execution's
        outputs threaded in as the next one's output-seed operands (data
        dependency serializes them on device); blocks once at the end.
        Wall-time slope over n_chain isolates on-device execution time from
        per-call host/RPC overhead."""
        import jax
        from jax.experimental.shard_map import shard_map
        from jax.sharding import PartitionSpec

        n_params = len(self.in_names)
        in_specs = (PartitionSpec("core"),) * (n_params + len(self.out_names))
        out_specs = (PartitionSpec("core"),) * len(self.out_names)
        fn = jax.jit(
            shard_map(
                self._body,
                mesh=self.mesh,
                in_specs=in_specs,
                out_specs=out_specs,
                check_rep=False,
            ),
            keep_unused=True,
        )
        dev_zeros = [self.jax.device_put(z, self.sharding) for z in self.zero_outs]

        def run():
            outs = tuple(dev_zeros)
            for _ in range(n_chain):
                outs = fn(*self.dev_in, *outs)
            return self.jax.block_until_ready(outs)

        return run


_RUNNER_CACHE = {}


def _get_runner(S, kind="main"):
    key = (S, kind)
    if key not in _RUNNER_CACHE:
        nc = build_nc(S) if kind == "main" else build_noop_nc(S)
        _RUNNER_CACHE[key] = Runner(nc)
    return _RUNNER_CACHE[key]


def build_noop_nc(S):
    """Same I/O signature, near-zero device work — for launch-overhead calibration."""
    nc = bass.Bass()
    nc.declare_dram_parameter("feats", [B_LOC, S, T], F32, isOutput=False)
    consts_d = nc.declare_dram_parameter("consts", [B_LOC, NCONST], F32, isOutput=False)
    path_d = nc.declare_dram_parameter("path", [32, S * 4], F32, isOutput=True)
    with TileContext(nc) as tc:
        with tc.tile_pool(name="w", bufs=1) as pool:
            t = pool.tile([32, T], F32)
            nc.sync.dma_start(out=t[:], in_=consts_d[0:32, 0:T])
            ti = pool.tile([32, T], F32)
            nc.vector.tensor_copy(ti[:], t[:])
            nc.sync.dma_start(out=path_d[:, 0:T], in_=ti[:])
    _strip_redundant_pe_waits(nc)
    return nc


def _in_maps_for(feats, transitions, start_transitions, stop_transitions, n_cores):
    consts = _make_const_inputs(transitions, start_transitions, stop_transitions)
    in_maps = []
    for c in range(n_cores):
        m = dict(consts)
        m["feats"] = np.ascontiguousarray(feats[c * B_LOC : (c + 1) * B_LOC])
        in_maps.append(m)
    return in_maps


def run_on_cores(feats, transitions, start_transitions, stop_transitions, trace=False):
    feats = np.asarray(feats, dtype=np.float32)
    B, S, T_ = feats.shape
    assert T_ == T and B % B_LOC == 0
    n_cores = B // B_LOC
    runner = _get_runner(S)
    runner.set_inputs(
        _in_maps_for(feats, transitions, start_transitions, stop_transitions, n_cores)
    )
    out = runner.execute()["path"]
    # device layout: per core [32, S, 4] with path[32*k + b', s] = out[b', s, k]
    out = out.reshape(n_cores, 32, S, 4).transpose(0, 3, 1, 2)
    return np.ascontiguousarray(out).reshape(B, S).astype(np.int32), None


def kernel(feats, tags, transitions, start_transitions, stop_transitions):
    out, _ = run_on_cores(feats, transitions, start_transitions, stop_transitions)
    return out



# revision 53
# speedup vs baseline: 1.0029x; 1.0029x over previous
"""Viterbi decode (CRF) kernel for Trainium2, data-parallel over batch on 8 cores.

Split-direction Viterbi: a forward max-plus pass over positions 0..M and a
backward pass over S-1..M run concurrently (two independent carry chains), then
the best meeting tag at M is picked and TWO independent backtrace chains walk
outward (M->0 and M->S-1), interleaved to hide each other's latency.

The per-step broadcast-add + segmented max-reduce is fused into ONE custom DVE
instruction (ADD_SEGMAX): a MAX-scan over (carry[b,q] + trans[q,c]) whose
running value is re-seeded at each 32-element page boundary via a hand-built
SUB_DIM_DONE step state; the per-page max is read back from the last element
of each page. The feat-add runs on GPSIMD, off the DVE critical path.

The backtrace recomputes the argmax only along the traced path: the needed
trans column/row is selected by 4 concurrent 32x32 PE matmuls on a one-hot,
then a fused add+max custom DVE op (ADD_MAXREDUCE) + MaxIndex give the tag.
"""

import dataclasses as _dc
import sys

sys.path.insert(0, "/opt/trn_rl_repo")

import numpy as np

from concourse import bass, mybir
from concourse import dve_ops as _dve_ops
from concourse.dve_ops import DveOp, _COMPILE_CACHE
from concourse.dve_spec import (
    AluOp as _AluOp,
    Latch as _Latch,
    MaxNeg as _MaxNeg,
    N_LANES as _N_LANES,
    N_STAGES as _N_STAGES,
    Scan as _Scan,
    Spec as _Spec,
    Src0 as _Src0,
    Src1 as _Src1,
    Trigger as _Trigger,
    _assemble,
    _build_placement,
    _build_state_machine,
    _collect,
    _hoist_stream_invariant_ops,
    _Stage,
    _validate_body,
    lower as _lower,
    scan as _scan,
)
from concourse.dve_uop import DveOpSpec
from concourse.tile import TileContext

F32 = mybir.dt.float32
I32 = mybir.dt.int32
U32 = mybir.dt.uint32

B_LOC = 128  # batch rows per core
T = 32  # tags
N_CORES = 8

# ---------------------------------------------------------------------------
# Custom DVE ops
# ---------------------------------------------------------------------------


def _segmax_ref(in0, in1, c0, c1, c2):
    x = in0.astype(np.float32) + in1.astype(np.float32)
    return np.maximum.accumulate(x, axis=-1)


def _addmax_ref(in0, in1, c0, c1, c2):
    x = in0.astype(np.float32) + in1.astype(np.float32)
    acc = x.reshape(x.shape[0], -1).max(axis=-1, keepdims=True)
    return x, acc


def _lower_segmax(spec, ver):
    """Stock [seed, steady] machine plus a SUB_DIM_DONE step state that
    re-seeds the MAX-scan (max(-FLT_MAX, expr) = expr) while consuming the
    first element of each page -> per-page segmented running max."""
    n_lanes, n_stages = _N_LANES[ver], _N_STAGES[ver]
    _validate_body(spec, ver)
    spec = _hoist_stream_invariant_ops(spec)
    scans = _collect(spec.body, _Scan)
    latches = _collect(spec.body, _Latch)
    assert len(scans) == 1 and not latches
    p = _build_placement(spec, scans, n_stages, n_lanes)
    states = _build_state_machine(spec, scans, latches, p)
    assert len(states) == 2, f"expected [seed, steady], got {len(states)}"
    seed, steady = states
    sc = scans[0]
    d = p.node_stage[sc]
    steady2 = _dc.replace(
        steady,
        trigger=(_Trigger.SRC_TENSOR_DONE, _Trigger.SUB_DIM_DONE, _Trigger.NONE),
        next=(0, 2, 0),
    )
    step = _dc.replace(
        steady,
        overrides={**steady.overrides, d: _Stage(_AluOp.MAX, _MaxNeg, sc.expr)},
        trigger=(_Trigger.SRC_TENSOR_DONE, _Trigger.SUB_DIM_DONE, _Trigger.COUNT),
        next=(0, 2, 1),
        repeat=1,
    )
    out = [_assemble(s) for s in (seed, steady2, step)]
    for u in out:
        u.validate(ver)
    return out


@_dc.dataclass(frozen=True)
class _HandOp(DveOp):
    """DveOp compiled by a custom lowering; sha pin skipped (computed live)."""

    def compile(self, ver):
        key = (self.name, ver)
        if (r := _COMPILE_CACHE.get(key)) is not None:
            return r
        uops = (
            _lower_segmax(self.spec, ver)
            if self.name == "ANT_ADD_SEGMAX"
            else _lower(self.spec, ver=ver)
        )
        result = DveOpSpec(
            name=self.name,
            opcode=_dve_ops.get_dve_sub_opcode(self.name),
            uops=uops,
            rd1_en=True,
        )
        _COMPILE_CACHE[key] = result
        return result


ADD_SEGMAX = _HandOp(
    "ANT_ADD_SEGMAX",
    _Spec(body=_scan(_AluOp.MAX, _Src0 + _Src1, init=_MaxNeg), reference=_segmax_ref),
    subdim=True,
    uops_sha={},
)

ADD_MAXREDUCE = _HandOp(
    "ANT_ADD_MAXREDUCE",
    _Spec(body=_Src0 + _Src1, accum=_AluOp.MAX, reference=_addmax_ref),
    subdim=False,
    uops_sha={},
)


def _register(op):
    if op.name in _dve_ops._SUB_OPCODE_FOR_NAME:
        return
    _dve_ops.OPS.append(op)
    _dve_ops._SUB_OPCODE_FOR_NAME[op.name] = (
        _dve_ops._CUSTOM_DVE_ROW_BASE + len(_dve_ops.OPS) - 1
    )
    _dve_ops.CUSTOM_DVE_SPECS[op.name] = op.spec
    assert max(_dve_ops._SUB_OPCODE_FOR_NAME.values()) < 0x20


_register(ADD_SEGMAX)
_register(ADD_MAXREDUCE)

# ---------------------------------------------------------------------------
# Kernel build
# ---------------------------------------------------------------------------

NCONST = 2 * T * T + 7 * T  # transT, transB, transmmF, transmmB, start, stop, idxW, iota2


def build_nc(S: int, fix_waits: bool = True):
    M = S // 2
    nc = bass.Bass()

    feats_d = nc.declare_dram_parameter("feats", [B_LOC, S, T], F32, isOutput=False)
    consts_d = nc.declare_dram_parameter("consts", [B_LOC, NCONST], F32, isOutput=False)
    # path layout: [32, S, 4] flattened; path[32*k + b', s] = dram[b', s*4+k].
    # Kept fp32 end-to-end (host converts): HW tensor_copy fp32->int32
    # bitcasts rather than converts.
    path_d = nc.declare_dram_parameter("path", [32, S * 4], F32, isOutput=True)

    add = mybir.AluOpType.add
    iseq = mybir.AluOpType.is_equal

    with TileContext(nc) as tc:
        with (
            tc.tile_pool(name="const", bufs=1) as cpool,
            tc.tile_pool(name="featp", bufs=1) as fpool,
            tc.tile_pool(name="work", bufs=1) as wpool,
            tc.tile_pool(name="scr", bufs=2) as spool,
            tc.tile_pool(name="psum", bufs=2, space="PSUM") as ppool,
            tc.tile_pool(name="psumIdx", bufs=1, space="PSUM") as ipool,
        ):
            consts_t = cpool.tile([B_LOC, NCONST], F32)
            nc.sync.dma_start(out=consts_t[:], in_=consts_d[:])
            o = 0
            transT3 = consts_t[:, o : o + T * T].rearrange("p (c q) -> p c q", q=T)
            o += T * T
            transB3 = consts_t[:, o : o + T * T].rearrange("p (m r) -> p m r", r=T)
            o += T * T
            transmmF = consts_t[:, o : o + T]
            o += T
            transmmB = consts_t[:, o : o + T]
            o += T
            start_t = consts_t[:, o : o + T]
            o += T
            stop_t = consts_t[:, o : o + T]
            o += T
            # idxW[p, k] = p%32 if p//32==k else 0: one PE matmul with lhsT =
            # the block-diagonal one-hot ohT gives FpIdx[b', k] = argmax tag
            # of batch row 32k+b' (the path value), on 32 partitions x 4.
            idxW = consts_t[:, o : o + 4]
            o += T
            iota2_t = consts_t[:, o : o + 2 * T]

            # ft arrives in interleaved lo/hi chunks on the ACT engine's DMA
            # queue (serial per queue, concurrent with the consts DMA on SP's)
            # so both passes can start ~20us before the full tensor lands.
            ft = fpool.tile([B_LOC, S, T], F32)
            NCH = 4
            ck = S // (2 * NCH) if S >= 2 * NCH else S
            if ck < S:
                for k in range(NCH):
                    lo = k * ck
                    nc.scalar.dma_start(
                        out=ft[:, lo : lo + ck, :], in_=feats_d[:, lo : lo + ck, :]
                    )
                    hi = S - (k + 1) * ck
                    nc.scalar.dma_start(
                        out=ft[:, hi : hi + ck, :], in_=feats_d[:, hi : hi + ck, :]
                    )
            else:
                nc.scalar.dma_start(out=ft[:], in_=feats_d[:])

            carF = wpool.tile([B_LOC, M + 1, T], F32)
            vpB = wpool.tile([B_LOC, M, T], F32)  # vpB[:, j] = vplus_{M+1+j}
            path_t = wpool.tile([32, S * 4], F32)

            # one DMA-wait touch per (engine, DMA) pair; later consumers
            # inherit via engine program order.
            tt0 = wpool.tile([B_LOC, 1], F32, tag="touch0")
            nc.vector.tensor_copy(tt0[:], consts_t[:, 0:1])
            tt1 = wpool.tile([B_LOC, 1], F32, tag="touch1")
            nc.vector.tensor_copy(tt1[:], ft[:, 0, 0:1])
            tt2 = wpool.tile([B_LOC, 1], F32, tag="touch2")
            nc.gpsimd.tensor_copy(tt2[:], consts_t[:, 0:1])
            tt3 = wpool.tile([B_LOC, 1], F32, tag="touch3")
            nc.gpsimd.tensor_copy(tt3[:], ft[:, 0, 0:1])
            tt4 = wpool.tile([B_LOC, 1], F32, tag="touch4")
            nc.gpsimd.tensor_copy(tt4[:], ft[:, S - 1, 0:1])
            Fp0 = ppool.tile([B_LOC, T], F32, tag="FpF")
            nc.tensor.matmul(
                Fp0[0:32, :],
                transmmF[0:32, :],
                transmmF[0:32, :],
                start=True,
                stop=True,
                tile_position=(0, 0),
            )

            # ---------------- forward + backward passes ----------------
            nc.vector.tensor_tensor(carF[:, 0, :], ft[:, 0, :], start_t, op=add)

            def seg_add_max(scr, trans3, vec):
                vb = vec.unsqueeze(1).broadcast_to([B_LOC, T, T])
                nc.vector._custom_dve(ADD_SEGMAX, out=scr[:], in0=trans3, in1=vb)

            # round r: segB_r consumes vplus_{S-r} -> b_{S-1-r};
            #          segF_r consumes carF[r-1] -> carF[r]
            prevB = None  # scratchB holding b_{S-r} page-maxes at [:, :, T-1]
            for r in range(1, M + 1):
                if ck < S and r > 1 and (r - 1) % ck == 0:
                    # next ft lo/hi chunk pair: one Pool-side DMA-wait touch each
                    k = (r - 1) // ck
                    ttl = wpool.tile([B_LOC, 1], F32, tag=f"tchlo{k}")
                    nc.gpsimd.tensor_copy(ttl[:], ft[:, k * ck, 0:1])
                    tth = wpool.tile([B_LOC, 1], F32, tag=f"tchhi{k}")
                    nc.gpsimd.tensor_copy(tth[:], ft[:, S - (k + 1) * ck, 0:1])
                if r <= M - 1:
                    # featB: vplus_{S-r} = b_{S-r} + ft[S-r]
                    j = S - r - (M + 1)  # vpB slot for position S-r
                    bprev = stop_t if prevB is None else prevB[:, :, T - 1]
                    nc.gpsimd.tensor_tensor(vpB[:, j, :], bprev, ft[:, S - r, :], op=add)
                    scrB = spool.tile([B_LOC, T, T], F32, tag="scrB")
                    seg_add_max(scrB, transB3, vpB[:, j, :])
                    prevB = scrB
                scrF = spool.tile([B_LOC, T, T], F32, tag="scrF")
                seg_add_max(scrF, transT3, carF[:, r - 1, :])
                nc.gpsimd.tensor_tensor(
                    carF[:, r, :], scrF[:, :, T - 1], ft[:, r, :], op=add
                )

            # ---------------- stitch ----------------
            # One-hot of the meeting tag comes straight from is_equal(tot, max)
            # (no MaxIndex): fp32 sums of distinct random values tie with
            # negligible probability, and the reduce's accumulator sees the
            # exact same fp32 values it writes, so eq(tot, mx) is exact.
            tot = wpool.tile([B_LOC, T], F32)
            mxS = wpool.tile([B_LOC, 1], F32)
            nc.vector._custom_dve(
                ADD_MAXREDUCE,
                out=tot[:],
                in0=carF[:, M, :],
                in1=prevB[:, :, T - 1],
                accum_out=mxS[:],
            )

            # ---------------- backtrace (two independent chains) ----------------
            # Per chain step (all-DVE serial segment, then one PE hop):
            #   ADD_MAXREDUCE(tot = scores + gathered trans column, mx)
            #   is_equal(oh = tot == mx)           <- next step's one-hot
            #   transpose(ohT, 32x32 blocks)
            #   4x PE matmul: moving operand carries trans AND an iota column,
            #     so Fp[:, T] is the argmax INDEX (the path tag) directly.
            #   ACT copies Fp[:, T] into the int32 path tile (cast), off the
            #   critical chain.
            # One-hot buffers are 2T wide with the upper half zeroed once
            # (matches the baseline's proven StreamTranspose shape): eq writes
            # cols 0:T, the transpose covers 2T, PE reads cols 0:T of the
            # transposed result. No buffer aliasing: the scheduler's natural
            # breadth-first interleave [MR_F,MR_B,eq_F,eq_B,TR_F,TR_B] puts
            # one instruction between every same-engine RAW pair, letting the
            # wait-strip pass remove ALL the per-step self-semaphores (and
            # their completion-ack stalls) safely.
            bufF = wpool.tile([B_LOC, 2 * T], F32)
            bufB = wpool.tile([B_LOC, 2 * T], F32)
            ohTF = wpool.tile([B_LOC, 2 * T], F32)
            ohTB = wpool.tile([B_LOC, 2 * T], F32)
            nc.gpsimd.memset(bufF[:, T:], 0.0)
            nc.gpsimd.memset(bufB[:, T:], 0.0)
            ohF = bufF[:, 0:T]
            ohB = bufB[:, 0:T]
            totF = wpool.tile([B_LOC, T], F32)
            totB = wpool.tile([B_LOC, T], F32)
            mxF = wpool.tile([B_LOC, 1], F32)
            mxB = wpool.tile([B_LOC, 1], F32)

            # ACT touches every ft chunk + consts once (cheap, long before
            # phase 2 needs them): the final path DMA's queue-order wait then
            # becomes transitively implied by its ACT-completion wait and is
            # stripped. Allocated after the backtrace work tiles so pool-slot
            # reuse deps don't attach ACT waits to backtrace instructions.
            if ck < S:
                for k in range(NCH):
                    ttA = wpool.tile([B_LOC, 1], F32, tag=f"touchA{k}")
                    nc.scalar.copy(ttA[:], ft[:, k * ck, 0:1])
                    ttB = wpool.tile([B_LOC, 1], F32, tag=f"touchB{k}")
                    nc.scalar.copy(ttB[:], ft[:, S - (k + 1) * ck, 0:1])
            else:
                ttA = wpool.tile([B_LOC, 1], F32, tag="touchA")
                nc.scalar.copy(ttA[:], ft[:, 0, 0:1])
            ttC = wpool.tile([B_LOC, 1], F32, tag="touchC")
            nc.scalar.copy(ttC[:], consts_t[:, 0:1])

            def oh_transpose(oh, buf, ohT, tot_t, mx_t):
                # tensor_scalar with a per-partition [p,1] scalar operand: a
                # stride-0 broadcast on tensor_tensor silently reads garbage
                # on hardware (CoreSim models it fine — verified empirically).
                nc.vector.tensor_scalar(oh, tot_t, mx_t[:, 0:1], None, op0=iseq)
                nc.vector.transpose(ohT[:], buf[:])

            # path values accumulate directly in a persistent PSUM buffer
            # [32, S, 4] (block-transposed batch layout, see idxW): the idx
            # matmul of each backtrace step writes its 4-float result at its
            # position — no per-step copy-out instruction, no WAR hazards
            # (each position is written once), nothing but PE touches it until
            # the final copy. Host code undoes the layout.
            pathP = ipool.tile([32, S, 4], F32)

            def pe_select(ohT, transmm, tag, pos):
                # idx matmul FIRST so the chain's maxreduce (which waits on
                # the last select) transitively covers every reader of ohT —
                # the wait-strip pass needs that to resolve the transpose's
                # WAR wait. pos=None skips it (stitch emits the meeting tag
                # once, via the F-side call).
                if pos is not None:
                    nc.tensor.matmul(
                        pathP[:, pos, :],
                        ohT[:, 0:T],
                        idxW,
                        start=True,
                        stop=True,
                        tile_position=(0, 0),
                    )
                Fp = ppool.tile([B_LOC, T], F32, tag=tag)
                for k in range(4):
                    nc.tensor.matmul(
                        Fp[32 * k : 32 * k + 32, :],
                        ohT[32 * k : 32 * k + 32, 0:T],
                        transmm[32 * k : 32 * k + 32, :],
                        start=True,
                        stop=True,
                        tile_position=(32 * k, 32 * k),
                    )
                return Fp

            def add_max(scores, Fp, tot_t, mx_t):
                nc.vector._custom_dve(
                    ADD_MAXREDUCE,
                    out=tot_t[:],
                    in0=scores,
                    in1=Fp[:],
                    accum_out=mx_t[:],
                )

            # both chains start from the same meeting one-hot. FpB is emitted
            # first so the F chain's first maxreduce waits on the HIGHEST PE
            # sem value, which lets the wait-strip pass resolve the round-0
            # transpose's PE WAR wait by transitivity.
            oh_transpose(ohF, bufF, ohTF, tot[:], mxS)
            FpB = pe_select(ohTF, transmmB, "FpB", None)
            FpF = pe_select(ohTF, transmmF, "FpF", M)

            # F chain: step t (t=0..M-1) consumes FpF for position i=M-t and
            # produces path[M-t-1]. B chain: step t (t=0..M-2) consumes FpB
            # for position i=M+t and produces path[M+t+1].
            for t in range(M):
                add_max(carF[:, M - t - 1, :], FpF, totF, mxF)
                oh_transpose(ohF, bufF, ohTF, totF[:], mxF)
                FpF = pe_select(ohTF, transmmF, "FpF", M - t - 1)
                if t <= M - 2:
                    add_max(vpB[:, t, :], FpB, totB, mxB)
                    oh_transpose(ohB, bufB, ohTB, totB[:], mxB)
                    FpB = pe_select(ohTB, transmmB, "FpB", M + t + 1)

            nc.vector.tensor_copy(path_t[:], pathP[:].rearrange("p a b -> p (a b)"))
            nc.sync.dma_start(out=path_d[:], in_=path_t[:])

    mybir.codegen_inst_isa_subclasses(nc)
    if fix_waits:
        _strip_redundant_pe_waits(nc)
    return nc


def _strip_redundant_pe_waits(nc):
    """Walrus encodes at most one sync-wait per compute instruction.

    1. Merge multiple waits on the same semaphore to the max value.
    2. Split multi-wait drains into chains of single-wait drains.
    3. Drop a non-DVE wait on a DVE-waiting instruction when some DVE
       instruction with completion tick <= the DVE wait value already waited
       on that semaphore >= the required value (transitive implication that
       Tile doesn't minimize across procs)."""
    f = nc.m.functions[0]
    insts = [i for blk in f.blocks for i in blk.instructions]

    from concourse import mybir as _mybir
    import copy as _copy

    # 1. same-sem merge
    for inst in insts:
        si = inst.sync_info
        if si is None or not si.on_wait or len(si.on_wait) <= 1:
            continue
        best = {}
        for w in si.on_wait:
            k = w.ant_name
            if k not in best or w.wait_value > best[k].wait_value:
                best[k] = w
        if len(best) < len(si.on_wait):
            inst.sync_info = _mybir.SyncInfo(
                on_wait=list(best.values()), on_update=list(si.on_update or [])
            )



    # 1b. drop same-engine self-waits whose producer has at least one
    # intervening instruction in the engine's own queue: in-order issue plus
    # the intervening op's execution time covers the producer's write-
    # visibility window. ADJACENT RAW pairs KEEP their sem: hardware makes an
    # op's writes visible ~95ns after its pipe drain, so the immediately
    # following op's reads can race them (observed empirically — a transpose
    # reading its is_equal predecessor's output returned stale data).
    own_count: dict = {}
    for inst in insts:
        si = inst.sync_info
        eng = str(inst.engine).split(".")[-1]
        if si is not None and si.on_wait:
            keep = [
                w
                for w in si.on_wait
                if not (
                    w.ant_name
                    and w.ant_name.split("_")[0] == eng
                    and w.wait_value <= own_count.get(w.ant_name, 0) - 1
                )
            ]
            if len(keep) < len(si.on_wait):
                inst.sync_info = _mybir.SyncInfo(
                    on_wait=keep, on_update=list(si.on_update or [])
                )
        si = inst.sync_info
        if si is not None:
            for u in si.on_update or []:
                if u.ant_name:
                    own_count[u.ant_name] = own_count.get(u.ant_name, 0) + u.update_value

    # cumulative: after the k-th E-sem increment, the largest value of each
    # other semaphore that engine E has (transitively) waited on so far.
    def prefix(name):
        return name.split("_")[0]

    tick = {}  # sem prefix -> increments so far
    cur_max = {}  # engine prefix -> {other sem prefix -> max waited value}
    observed = {}  # (engine prefix, other prefix) -> [(tick, maxval)]
    for inst in insts:
        si = inst.sync_info
        if si is None:
            continue
        eng = str(inst.engine).split(".")[-1]
        cm = cur_max.setdefault(eng, {})
        for w in si.on_wait or []:
            if w.ant_name:
                p = prefix(w.ant_name)
                if p != eng:
                    cm[p] = max(cm.get(p, 0), w.wait_value)
        for u in si.on_update or []:
            if u.ant_name:
                p = prefix(u.ant_name)
                tick[p] = tick.get(p, 0) + u.update_value
                if p == eng:
                    for q, v in cm.items():
                        observed.setdefault((p, q), []).append((tick[p], v))

    def implied(via_prefix, via_val, other_name, other_val):
        """True when "via-sem >= via_val" transitively implies
        "other-sem >= other_val": the via engine had waited on other-sem
        >= other_val by the time it made its via_val-th increment."""
        p = prefix(other_name)
        best = 0
        for k, v in observed.get((via_prefix, p), []):
            if k <= via_val:
                best = max(best, v)
        return best >= other_val

    # 2. split multi-wait drains
    for blk in f.blocks:
        new_list = []
        for inst in blk.instructions:
            si = inst.sync_info
            if (
                type(inst).__name__ == "InstDrain"
                and si is not None
                and si.on_wait
                and len(si.on_wait) > 1
            ):
                waits = list(si.on_wait)
                for k, w in enumerate(waits[:-1]):
                    clone = _copy.copy(inst)
                    clone.name = f"{inst.name}-w{k}"
                    clone.sync_info = _mybir.SyncInfo(on_wait=[w], on_update=[])
                    new_list.append(clone)
                inst.sync_info = _mybir.SyncInfo(
                    on_wait=[waits[-1]], on_update=list(si.on_update or [])
                )
            new_list.append(inst)
        blk.instructions[:] = new_list

    # 3. transitivity strip: for each wait, check whether one of the OTHER
    # waits on the instruction already implies it; drop implied waits.
    for inst in insts:
        si = inst.sync_info
        if si is None or not si.on_wait or len(si.on_wait) <= 1:
            continue
        waits = list(si.on_wait)
        keep = []
        for i, w in enumerate(waits):
            redundant = any(
                implied(prefix(v.ant_name), v.wait_value, w.ant_name, w.wait_value)
                for j, v in enumerate(waits)
                if j != i and (v in keep or j > i)
            )
            if not redundant:
                keep.append(w)
        if len(keep) < len(waits):
            inst.sync_info = _mybir.SyncInfo(
                on_wait=keep, on_update=list(si.on_update or [])
            )

    # 4. fallback: split residual multi-wait compute instructions by hoisting
    # all but one wait onto single-wait drains inserted just before them on
    # the same engine (a satisfied drain retires in a few ns).
    n_split = 0
    for blk in f.blocks:
        new_list = []
        for inst in blk.instructions:
            si = inst.sync_info
            if (
                si is not None
                and si.on_wait
                and len(si.on_wait) > 1
                and type(inst).__name__
                not in ("InstDrain", "InstEventSemaphore", "InstISA", "InstCall")
            ):
                waits = list(si.on_wait)
                for k, w in enumerate(waits[:-1]):
                    d = _mybir.InstDrain(
                        name=f"{inst.name}-sw{k}",
                        ins=[],
                        outs=[],
                        bass_is_fusable=False,
                    )
                    d.engine = inst.engine
                    d.sync_info = _mybir.SyncInfo(on_wait=[w], on_update=[])
                    new_list.append(d)
                    n_split += 1
                inst.sync_info = _mybir.SyncInfo(
                    on_wait=[waits[-1]], on_update=list(si.on_update or [])
                )
            new_list.append(inst)
        blk.instructions[:] = new_list


def _make_const_inputs(transitions, start_transitions, stop_transitions):
    transitions = np.asarray(transitions, dtype=np.float32)
    start = np.asarray(start_transitions, dtype=np.float32)
    stop = np.asarray(stop_transitions, dtype=np.float32)
    consts = np.zeros((B_LOC, NCONST), dtype=np.float32)
    o = 0
    consts[:, o : o + T * T] = transitions.T.reshape(1, T * T)  # [c*T+q] = trans[q,c]
    o += T * T
    consts[:, o : o + T * T] = transitions.reshape(1, T * T)  # [m*T+r] = trans[m,r]
    o += T * T
    consts[:, o : o + T] = np.tile(transitions.T, (4, 1))  # transmmF[p,f]=trans[f,p%32]
    o += T
    consts[:, o : o + T] = np.tile(transitions, (4, 1))  # transmmB[p,f]=trans[p%32,f]
    o += T
    consts[:, o : o + T] = start[None, :]
    o += T
    consts[:, o : o + T] = stop[None, :]
    o += T
    # idxW[p, k] = p%32 if p//32==k else 0 (block-selective iota for the
    # path-index PE matmul); rest of the slot unused.
    p = np.arange(4 * T)
    consts[:, o : o + 4] = ((p[:, None] // T) == np.arange(4)[None, :]) * (
        p[:, None] % T
    ).astype(np.float32)
    o += T
    consts[:, o : o + 2 * T] = np.tile(np.arange(T, dtype=np.float32), 2)[None, :]
    return {"consts": consts}


class Runner:
    """Compile once, keep inputs device-resident, execute repeatedly."""

    def __init__(self, nc, n_cores=N_CORES):
        import jax
        from jax.sharding import Mesh, PartitionSpec, NamedSharding
        from jax.experimental.shard_map import shard_map
        from concourse import bass2jax

        self.jax = jax
        bass2jax.install_neuronx_cc_hook()

        partition_name = (
            nc.partition_id_tensor.name if nc.partition_id_tensor else None
        )
        in_names, out_names, out_avals, zero_outs = [], [], [], []
        for alloc in nc.m.functions[0].allocations:
            if not isinstance(alloc, mybir.MemoryLocationSet):
                continue
            name = alloc.memorylocations[0].name
            if alloc.kind == "ExternalInput":
                if name != partition_name:
                    in_names.append(name)
            elif alloc.kind == "ExternalOutput":
                out_names.append(name)
                shape = tuple(alloc.tensor_shape)
                dtype = mybir.dt.np(alloc.dtype)
                out_avals.append(jax.core.ShapedArray(shape, dtype))
                zero_outs.append(np.zeros((n_cores * shape[0], *shape[1:]), dtype))
        n_params = len(in_names)
        all_names = in_names + out_names
        if partition_name is not None:
            all_names = all_names + [partition_name]

        def _body(*args):
            operands = list(args)
            if partition_name is not None:
                operands.append(bass2jax.partition_id_tensor())
            outs = bass2jax._bass_exec_p.bind(
                *operands,
                out_avals=tuple(out_avals),
                in_names=tuple(all_names),
                out_names=tuple(out_names),
                lowering_input_output_aliases=(),
                sim_require_finite=True,
                sim_require_nnan=True,
                nc=nc,
            )
            return tuple(outs)

        self._body = _body
        devices = jax.devices()[:n_cores]
        assert len(devices) == n_cores
        self.mesh = Mesh(np.asarray(devices), ("core",))
        in_specs = (PartitionSpec("core"),) * (n_params + len(out_names))
        out_specs = (PartitionSpec("core"),) * len(out_names)
        self.sharded = jax.jit(
            shard_map(
                _body,
                mesh=self.mesh,
                in_specs=in_specs,
                out_specs=out_specs,
                check_rep=False,
            ),
            donate_argnums=tuple(range(n_params, n_params + len(out_names))),
            keep_unused=True,
        )
        self.sharding = NamedSharding(self.mesh, PartitionSpec("core"))
        self.in_names = in_names
        self.out_names = out_names
        self.out_avals = out_avals
        self.zero_outs = zero_outs
        self.n_cores = n_cores
        self.dev_in = None

    def set_inputs(self, in_maps):
        concat = [
            np.concatenate([np.asarray(m[name]) for m in in_maps], axis=0)
            for name in self.in_names
        ]
        self.dev_in = [self.jax.device_put(a, self.sharding) for a in concat]

    def execute(self):
        outs = self.sharded(*self.dev_in, *[z.copy() for z in self.zero_outs])
        outs = self.jax.block_until_ready(outs)
        return {
            name: np.asarray(outs[i]).reshape(
                self.n_cores, *self.out_avals[i].shape
            )
            for i, name in enumerate(self.out_names)
        }

    def make_chained(self, n_chain):
        """Callable dispatching the NEFF n_chain times, each execution's
        outputs threaded in as the next one's output-seed operands (data
        dependency serializes them on device); blocks once at the end.
        Wall-time slope over n_chain isolates on-device execution time from
        per-call host/RPC overhead."""
        import jax
        from jax.experimental.shard_map import shard_map
        from jax.sharding import PartitionSpec

        n_params = len(self.in_names)
        in_specs = (PartitionSpec("core"),) * (n_params + len(self.out_names))
        out_specs = (PartitionSpec("core"),) * len(self.out_names)
        fn = jax.jit(
            shard_map(
                self._body,
                mesh=self.mesh,
                in_specs=in_specs,
                out_specs=out_specs,
                check_rep=False,
            ),
            keep_unused=True,
        )
        dev_zeros = [self.jax.device_put(z, self.sharding) for z in self.zero_outs]

        def run():
            outs = tuple(dev_zeros)
            for _ in range(n_chain):
                outs = fn(*self.dev_in, *outs)
            return self.jax.block_until_ready(outs)

        return run


_RUNNER_CACHE = {}


def _get_runner(S, kind="main"):
    key = (S, kind)
    if key not in _RUNNER_CACHE:
        nc = build_nc(S) if kind == "main" else build_noop_nc(S)
        _RUNNER_CACHE[key] = Runner(nc)
    return _RUNNER_CACHE[key]


def build_noop_nc(S):
    """Same I/O signature, near-zero device work — for launch-overhead calibration."""
    nc = bass.Bass()
    nc.declare_dram_parameter("feats", [B_LOC, S, T], F32, isOutput=False)
    consts_d = nc.declare_dram_parameter("consts", [B_LOC, NCONST], F32, isOutput=False)
    path_d = nc.declare_dram_parameter("path", [32, S * 4], F32, isOutput=True)
    with TileContext(nc) as tc:
        with tc.tile_pool(name="w", bufs=1) as pool:
            t = pool.tile([32, T], F32)
            nc.sync.dma_start(out=t[:], in_=consts_d[0:32, 0:T])
            ti = pool.tile([32, T], F32)
            nc.vector.tensor_copy(ti[:], t[:])
            nc.sync.dma_start(out=path_d[:, 0:T], in_=ti[:])
    _strip_redundant_pe_waits(nc)
    return nc


def _in_maps_for(feats, transitions, start_transitions, stop_transitions, n_cores):
    consts = _make_const_inputs(transitions, start_transitions, stop_transitions)
    in_maps = []
    for c in range(n_cores):
        m = dict(consts)
        m["feats"] = np.ascontiguousarray(feats[c * B_LOC : (c + 1) * B_LOC])
        in_maps.append(m)
    return in_maps


def run_on_cores(feats, transitions, start_transitions, stop_transitions, trace=False):
    feats = np.asarray(feats, dtype=np.float32)
    B, S, T_ = feats.shape
    assert T_ == T and B % B_LOC == 0
    n_cores = B // B_LOC
    runner = _get_runner(S)
    runner.set_inputs(
        _in_maps_for(feats, transitions, start_transitions, stop_transitions, n_cores)
    )
    out = runner.execute()["path"]
    # device layout: per core [32, S, 4] with path[32*k + b', s] = out[b', s, k]
    out = out.reshape(n_cores, 32, S, 4).transpose(0, 3, 1, 2)
    return np.ascontiguousarray(out).reshape(B, S).astype(np.int32), None


def kernel(feats, tags, transitions, start_transitions, stop_transitions):
    out, _ = run_on_cores(feats, transitions, start_transitions, stop_transitions)
    return out



# revision 54
# speedup vs baseline: 1.1035x; 1.1003x over previous
"""Viterbi decode (CRF) kernel for Trainium2, data-parallel over batch on 8 cores.

Split-direction Viterbi: a forward max-plus pass over positions 0..M and a
backward pass over S-1..M run concurrently (two independent carry chains), then
the best meeting tag at M is picked and TWO independent backtrace chains walk
outward (M->0 and M->S-1), interleaved to hide each other's latency.

The per-step broadcast-add + segmented max-reduce is fused into ONE custom DVE
instruction (ADD_SEGMAX): a MAX-scan over (carry[b,q] + trans[q,c]) whose
running value is re-seeded at each 32-element page boundary via a hand-built
SUB_DIM_DONE step state; the per-page max is read back from the last element
of each page. The feat-add runs on GPSIMD, off the DVE critical path.

The backtrace recomputes the argmax only along the traced path: the needed
trans column/row is selected by 4 concurrent 32x32 PE matmuls on a one-hot,
then a fused add+max custom DVE op (ADD_MAXREDUCE) + MaxIndex give the tag.
"""

import dataclasses as _dc
import sys

sys.path.insert(0, "/opt/trn_rl_repo")

import numpy as np

from concourse import bass, mybir
from concourse import dve_ops as _dve_ops
from concourse.dve_ops import DveOp, _COMPILE_CACHE
from concourse.dve_spec import (
    AluOp as _AluOp,
    Latch as _Latch,
    MaxNeg as _MaxNeg,
    N_LANES as _N_LANES,
    N_STAGES as _N_STAGES,
    Scan as _Scan,
    Spec as _Spec,
    Src0 as _Src0,
    Src1 as _Src1,
    Trigger as _Trigger,
    _assemble,
    _build_placement,
    _build_state_machine,
    _collect,
    _hoist_stream_invariant_ops,
    _Stage,
    _validate_body,
    lower as _lower,
    scan as _scan,
)
from concourse.dve_uop import DveOpSpec
from concourse.tile import TileContext

F32 = mybir.dt.float32
I32 = mybir.dt.int32
U32 = mybir.dt.uint32

B_LOC = 128  # batch rows per core
T = 32  # tags
N_CORES = 8

# ---------------------------------------------------------------------------
# Custom DVE ops
# ---------------------------------------------------------------------------


def _segmax_ref(in0, in1, c0, c1, c2):
    x = in0.astype(np.float32) + in1.astype(np.float32)
    return np.maximum.accumulate(x, axis=-1)


def _addmax_ref(in0, in1, c0, c1, c2):
    x = in0.astype(np.float32) + in1.astype(np.float32)
    acc = x.reshape(x.shape[0], -1).max(axis=-1, keepdims=True)
    return x, acc


def _lower_segmax(spec, ver):
    """Stock [seed, steady] machine plus a SUB_DIM_DONE step state that
    re-seeds the MAX-scan (max(-FLT_MAX, expr) = expr) while consuming the
    first element of each page -> per-page segmented running max."""
    n_lanes, n_stages = _N_LANES[ver], _N_STAGES[ver]
    _validate_body(spec, ver)
    spec = _hoist_stream_invariant_ops(spec)
    scans = _collect(spec.body, _Scan)
    latches = _collect(spec.body, _Latch)
    assert len(scans) == 1 and not latches
    p = _build_placement(spec, scans, n_stages, n_lanes)
    states = _build_state_machine(spec, scans, latches, p)
    assert len(states) == 2, f"expected [seed, steady], got {len(states)}"
    seed, steady = states
    sc = scans[0]
    d = p.node_stage[sc]
    steady2 = _dc.replace(
        steady,
        trigger=(_Trigger.SRC_TENSOR_DONE, _Trigger.SUB_DIM_DONE, _Trigger.NONE),
        next=(0, 2, 0),
    )
    step = _dc.replace(
        steady,
        overrides={**steady.overrides, d: _Stage(_AluOp.MAX, _MaxNeg, sc.expr)},
        trigger=(_Trigger.SRC_TENSOR_DONE, _Trigger.SUB_DIM_DONE, _Trigger.COUNT),
        next=(0, 2, 1),
        repeat=1,
    )
    out = [_assemble(s) for s in (seed, steady2, step)]
    for u in out:
        u.validate(ver)
    return out


@_dc.dataclass(frozen=True)
class _HandOp(DveOp):
    """DveOp compiled by a custom lowering; sha pin skipped (computed live)."""

    def compile(self, ver):
        key = (self.name, ver)
        if (r := _COMPILE_CACHE.get(key)) is not None:
            return r
        uops = (
            _lower_segmax(self.spec, ver)
            if self.name == "ANT_ADD_SEGMAX"
            else _lower(self.spec, ver=ver)
        )
        result = DveOpSpec(
            name=self.name,
            opcode=_dve_ops.get_dve_sub_opcode(self.name),
            uops=uops,
            rd1_en=True,
        )
        _COMPILE_CACHE[key] = result
        return result


ADD_SEGMAX = _HandOp(
    "ANT_ADD_SEGMAX",
    _Spec(body=_scan(_AluOp.MAX, _Src0 + _Src1, init=_MaxNeg), reference=_segmax_ref),
    subdim=True,
    uops_sha={},
)

ADD_MAXREDUCE = _HandOp(
    "ANT_ADD_MAXREDUCE",
    _Spec(body=_Src0 + _Src1, accum=_AluOp.MAX, reference=_addmax_ref),
    subdim=False,
    uops_sha={},
)


def _register(op):
    if op.name in _dve_ops._SUB_OPCODE_FOR_NAME:
        return
    _dve_ops.OPS.append(op)
    _dve_ops._SUB_OPCODE_FOR_NAME[op.name] = (
        _dve_ops._CUSTOM_DVE_ROW_BASE + len(_dve_ops.OPS) - 1
    )
    _dve_ops.CUSTOM_DVE_SPECS[op.name] = op.spec
    assert max(_dve_ops._SUB_OPCODE_FOR_NAME.values()) < 0x20


_register(ADD_SEGMAX)
_register(ADD_MAXREDUCE)

# ---------------------------------------------------------------------------
# Kernel build
# ---------------------------------------------------------------------------

NCONST = 2 * T * T + 7 * T  # transT, transB, transmmF, transmmB, start, stop, idxW, iota2


def build_nc(S: int, fix_waits: bool = True):
    M = S // 2
    nc = bass.Bass()

    feats_d = nc.declare_dram_parameter("feats", [B_LOC, S, T], F32, isOutput=False)
    consts_d = nc.declare_dram_parameter("consts", [B_LOC, NCONST], F32, isOutput=False)
    # path layout: [32, S, 4] flattened; path[32*k + b', s] = dram[b', s*4+k].
    # Kept fp32 end-to-end (host converts): HW tensor_copy fp32->int32
    # bitcasts rather than converts.
    path_d = nc.declare_dram_parameter("path", [32, S * 4], F32, isOutput=True)

    add = mybir.AluOpType.add
    iseq = mybir.AluOpType.is_equal

    with TileContext(nc) as tc:
        with (
            tc.tile_pool(name="const", bufs=1) as cpool,
            tc.tile_pool(name="featp", bufs=1) as fpool,
            tc.tile_pool(name="work", bufs=1) as wpool,
            tc.tile_pool(name="scr", bufs=2) as spool,
            tc.tile_pool(name="psum", bufs=2, space="PSUM") as ppool,
            tc.tile_pool(name="psumIdx", bufs=1, space="PSUM") as ipool,
        ):
            consts_t = cpool.tile([B_LOC, NCONST], F32)
            nc.sync.dma_start(out=consts_t[:], in_=consts_d[:])
            o = 0
            transT3 = consts_t[:, o : o + T * T].rearrange("p (c q) -> p c q", q=T)
            o += T * T
            transB3 = consts_t[:, o : o + T * T].rearrange("p (m r) -> p m r", r=T)
            o += T * T
            transmmF = consts_t[:, o : o + T]
            o += T
            transmmB = consts_t[:, o : o + T]
            o += T
            start_t = consts_t[:, o : o + T]
            o += T
            stop_t = consts_t[:, o : o + T]
            o += T
            # idxW[p, k] = p%32 if p//32==k else 0: one PE matmul with lhsT =
            # the block-diagonal one-hot ohT gives FpIdx[b', k] = argmax tag
            # of batch row 32k+b' (the path value), on 32 partitions x 4.
            idxW = consts_t[:, o : o + 4]
            o += T
            iota2_t = consts_t[:, o : o + 2 * T]

            # ft arrives in interleaved lo/hi chunks on the ACT engine's DMA
            # queue (serial per queue, concurrent with the consts DMA on SP's)
            # so both passes can start ~20us before the full tensor lands.
            ft = fpool.tile([B_LOC, S, T], F32)
            NCH = 4
            ck = S // (2 * NCH) if S >= 2 * NCH else S
            if ck < S:
                for k in range(NCH):
                    lo = k * ck
                    nc.scalar.dma_start(
                        out=ft[:, lo : lo + ck, :], in_=feats_d[:, lo : lo + ck, :]
                    )
                    hi = S - (k + 1) * ck
                    nc.scalar.dma_start(
                        out=ft[:, hi : hi + ck, :], in_=feats_d[:, hi : hi + ck, :]
                    )
            else:
                nc.scalar.dma_start(out=ft[:], in_=feats_d[:])

            carF = wpool.tile([B_LOC, M + 1, T], F32)
            vpB = wpool.tile([B_LOC, M, T], F32)  # vpB[:, j] = vplus_{M+1+j}
            path_t = wpool.tile([32, S * 4], F32)

            # one DMA-wait touch per (engine, DMA) pair; later consumers
            # inherit via engine program order.
            tt0 = wpool.tile([B_LOC, 1], F32, tag="touch0")
            nc.vector.tensor_copy(tt0[:], consts_t[:, 0:1])
            tt1 = wpool.tile([B_LOC, 1], F32, tag="touch1")
            nc.vector.tensor_copy(tt1[:], ft[:, 0, 0:1])
            tt2 = wpool.tile([B_LOC, 1], F32, tag="touch2")
            nc.gpsimd.tensor_copy(tt2[:], consts_t[:, 0:1])
            tt3 = wpool.tile([B_LOC, 1], F32, tag="touch3")
            nc.gpsimd.tensor_copy(tt3[:], ft[:, 0, 0:1])
            tt4 = wpool.tile([B_LOC, 1], F32, tag="touch4")
            nc.gpsimd.tensor_copy(tt4[:], ft[:, S - 1, 0:1])
            Fp0 = ppool.tile([B_LOC, T], F32, tag="FpF")
            nc.tensor.matmul(
                Fp0[0:32, :],
                transmmF[0:32, :],
                transmmF[0:32, :],
                start=True,
                stop=True,
                tile_position=(0, 0),
            )

            # ---------------- forward + backward passes ----------------
            nc.vector.tensor_tensor(carF[:, 0, :], ft[:, 0, :], start_t, op=add)

            def seg_add_max(scr, trans3, vec):
                vb = vec.unsqueeze(1).broadcast_to([B_LOC, T, T])
                nc.vector._custom_dve(ADD_SEGMAX, out=scr[:], in0=trans3, in1=vb)

            # round r: segB_r consumes vplus_{S-r} -> b_{S-1-r};
            #          segF_r consumes carF[r-1] -> carF[r]
            prevB = None  # scratchB holding b_{S-r} page-maxes at [:, :, T-1]
            for r in range(1, M + 1):
                if ck < S and r > 1 and (r - 1) % ck == 0:
                    # next ft lo/hi chunk pair: one Pool-side DMA-wait touch each
                    k = (r - 1) // ck
                    ttl = wpool.tile([B_LOC, 1], F32, tag=f"tchlo{k}")
                    nc.gpsimd.tensor_copy(ttl[:], ft[:, k * ck, 0:1])
                    tth = wpool.tile([B_LOC, 1], F32, tag=f"tchhi{k}")
                    nc.gpsimd.tensor_copy(tth[:], ft[:, S - (k + 1) * ck, 0:1])
                if r <= M - 1:
                    # featB: vplus_{S-r} = b_{S-r} + ft[S-r]
                    j = S - r - (M + 1)  # vpB slot for position S-r
                    bprev = stop_t if prevB is None else prevB[:, :, T - 1]
                    nc.gpsimd.tensor_tensor(vpB[:, j, :], bprev, ft[:, S - r, :], op=add)
                    scrB = spool.tile([B_LOC, T, T], F32, tag="scrB")
                    seg_add_max(scrB, transB3, vpB[:, j, :])
                    prevB = scrB
                scrF = spool.tile([B_LOC, T, T], F32, tag="scrF")
                seg_add_max(scrF, transT3, carF[:, r - 1, :])
                nc.gpsimd.tensor_tensor(
                    carF[:, r, :], scrF[:, :, T - 1], ft[:, r, :], op=add
                )

            # ---------------- stitch ----------------
            # One-hot of the meeting tag comes straight from is_equal(tot, max)
            # (no MaxIndex): fp32 sums of distinct random values tie with
            # negligible probability, and the reduce's accumulator sees the
            # exact same fp32 values it writes, so eq(tot, mx) is exact.
            tot = wpool.tile([B_LOC, T], F32)
            mxS = wpool.tile([B_LOC, 1], F32)
            nc.vector._custom_dve(
                ADD_MAXREDUCE,
                out=tot[:],
                in0=carF[:, M, :],
                in1=prevB[:, :, T - 1],
                accum_out=mxS[:],
            )

            # ---------------- backtrace (two independent chains) ----------------
            # Per chain step (all-DVE serial segment, then one PE hop):
            #   ADD_MAXREDUCE(tot = scores + gathered trans column, mx)
            #   is_equal(oh = tot == mx)           <- next step's one-hot
            #   transpose(ohT, 32x32 blocks)
            #   4x PE matmul: moving operand carries trans AND an iota column,
            #     so Fp[:, T] is the argmax INDEX (the path tag) directly.
            #   ACT copies Fp[:, T] into the int32 path tile (cast), off the
            #   critical chain.
            # No buffer aliasing: the scheduler's natural breadth-first
            # interleave [MR_F,MR_B,eq_F,eq_B,TR_F,TR_B] puts one instruction
            # between every same-engine RAW pair, letting the wait-strip pass
            # remove ALL the per-step self-semaphores (and their
            # completion-ack stalls) safely. [128, 32] fp32 StreamTranspose is
            # fine on HW once those sems are handled (probe-verified).
            bufF = wpool.tile([B_LOC, T], F32)
            bufB = wpool.tile([B_LOC, T], F32)
            ohTF = wpool.tile([B_LOC, T], F32)
            ohTB = wpool.tile([B_LOC, T], F32)
            ohF = bufF[:, 0:T]
            ohB = bufB[:, 0:T]
            totF = wpool.tile([B_LOC, T], F32)
            totB = wpool.tile([B_LOC, T], F32)
            mxF = wpool.tile([B_LOC, 1], F32)
            mxB = wpool.tile([B_LOC, 1], F32)

            # ACT touches every ft chunk + consts once (cheap, long before
            # phase 2 needs them): the final path DMA's queue-order wait then
            # becomes transitively implied by its ACT-completion wait and is
            # stripped. Allocated after the backtrace work tiles so pool-slot
            # reuse deps don't attach ACT waits to backtrace instructions.
            if ck < S:
                for k in range(NCH):
                    ttA = wpool.tile([B_LOC, 1], F32, tag=f"touchA{k}")
                    nc.scalar.copy(ttA[:], ft[:, k * ck, 0:1])
                    ttB = wpool.tile([B_LOC, 1], F32, tag=f"touchB{k}")
                    nc.scalar.copy(ttB[:], ft[:, S - (k + 1) * ck, 0:1])
            else:
                ttA = wpool.tile([B_LOC, 1], F32, tag="touchA")
                nc.scalar.copy(ttA[:], ft[:, 0, 0:1])
            ttC = wpool.tile([B_LOC, 1], F32, tag="touchC")
            nc.scalar.copy(ttC[:], consts_t[:, 0:1])

            def oh_transpose(oh, buf, ohT, tot_t, mx_t):
                # tensor_scalar with a per-partition [p,1] scalar operand: a
                # stride-0 broadcast on tensor_tensor silently reads garbage
                # on hardware (CoreSim models it fine — verified empirically).
                nc.vector.tensor_scalar(oh, tot_t, mx_t[:, 0:1], None, op0=iseq)
                nc.vector.transpose(ohT[:], buf[:])

            # path values accumulate directly in a persistent PSUM buffer
            # [32, S, 4] (block-transposed batch layout, see idxW): the idx
            # matmul of each backtrace step writes its 4-float result at its
            # position — no per-step copy-out instruction, no WAR hazards
            # (each position is written once), nothing but PE touches it until
            # the final copy. Host code undoes the layout.
            pathP = ipool.tile([32, S, 4], F32)

            def pe_select(ohT, transmm, tag, pos):
                # idx matmul FIRST so the chain's maxreduce (which waits on
                # the last select) transitively covers every reader of ohT —
                # the wait-strip pass needs that to resolve the transpose's
                # WAR wait. pos=None skips it (stitch emits the meeting tag
                # once, via the F-side call).
                if pos is not None:
                    nc.tensor.matmul(
                        pathP[:, pos, :],
                        ohT[:, 0:T],
                        idxW,
                        start=True,
                        stop=True,
                        tile_position=(0, 0),
                    )
                Fp = ppool.tile([B_LOC, T], F32, tag=tag)
                for k in range(4):
                    nc.tensor.matmul(
                        Fp[32 * k : 32 * k + 32, :],
                        ohT[32 * k : 32 * k + 32, 0:T],
                        transmm[32 * k : 32 * k + 32, :],
                        start=True,
                        stop=True,
                        tile_position=(32 * k, 32 * k),
                    )
                return Fp

            def add_max(scores, Fp, tot_t, mx_t):
                nc.vector._custom_dve(
                    ADD_MAXREDUCE,
                    out=tot_t[:],
                    in0=scores,
                    in1=Fp[:],
                    accum_out=mx_t[:],
                )

            # both chains start from the same meeting one-hot. FpB is emitted
            # first so the F chain's first maxreduce waits on the HIGHEST PE
            # sem value, which lets the wait-strip pass resolve the round-0
            # transpose's PE WAR wait by transitivity.
            oh_transpose(ohF, bufF, ohTF, tot[:], mxS)
            FpB = pe_select(ohTF, transmmB, "FpB", None)
            FpF = pe_select(ohTF, transmmF, "FpF", M)

            # F chain: step t (t=0..M-1) consumes FpF for position i=M-t and
            # produces path[M-t-1]. B chain: step t (t=0..M-2) consumes FpB
            # for position i=M+t and produces path[M+t+1].
            for t in range(M):
                add_max(carF[:, M - t - 1, :], FpF, totF, mxF)
                oh_transpose(ohF, bufF, ohTF, totF[:], mxF)
                FpF = pe_select(ohTF, transmmF, "FpF", M - t - 1)
                if t <= M - 2:
                    add_max(vpB[:, t, :], FpB, totB, mxB)
                    oh_transpose(ohB, bufB, ohTB, totB[:], mxB)
                    FpB = pe_select(ohTB, transmmB, "FpB", M + t + 1)

            nc.vector.tensor_copy(path_t[:], pathP[:].rearrange("p a b -> p (a b)"))
            nc.sync.dma_start(out=path_d[:], in_=path_t[:])

    mybir.codegen_inst_isa_subclasses(nc)
    if fix_waits:
        _strip_redundant_pe_waits(nc)
    return nc


def _strip_redundant_pe_waits(nc):
    """Walrus encodes at most one sync-wait per compute instruction.

    1. Merge multiple waits on the same semaphore to the max value.
    2. Split multi-wait drains into chains of single-wait drains.
    3. Drop a non-DVE wait on a DVE-waiting instruction when some DVE
       instruction with completion tick <= the DVE wait value already waited
       on that semaphore >= the required value (transitive implication that
       Tile doesn't minimize across procs)."""
    f = nc.m.functions[0]
    insts = [i for blk in f.blocks for i in blk.instructions]

    from concourse import mybir as _mybir
    import copy as _copy

    # 1. same-sem merge
    for inst in insts:
        si = inst.sync_info
        if si is None or not si.on_wait or len(si.on_wait) <= 1:
            continue
        best = {}
        for w in si.on_wait:
            k = w.ant_name
            if k not in best or w.wait_value > best[k].wait_value:
                best[k] = w
        if len(best) < len(si.on_wait):
            inst.sync_info = _mybir.SyncInfo(
                on_wait=list(best.values()), on_update=list(si.on_update or [])
            )



    # 1b. drop same-engine self-waits whose producer has at least one
    # intervening instruction in the engine's own queue: in-order issue plus
    # the intervening op's execution time covers the producer's write-
    # visibility window. ADJACENT RAW pairs KEEP their sem: hardware makes an
    # op's writes visible ~95ns after its pipe drain, so the immediately
    # following op's reads can race them (observed empirically — a transpose
    # reading its is_equal predecessor's output returned stale data).
    own_count: dict = {}
    for inst in insts:
        si = inst.sync_info
        eng = str(inst.engine).split(".")[-1]
        if si is not None and si.on_wait:
            keep = [
                w
                for w in si.on_wait
                if not (
                    w.ant_name
                    and w.ant_name.split("_")[0] == eng
                    and w.wait_value <= own_count.get(w.ant_name, 0) - 1
                )
            ]
            if len(keep) < len(si.on_wait):
                inst.sync_info = _mybir.SyncInfo(
                    on_wait=keep, on_update=list(si.on_update or [])
                )
        si = inst.sync_info
        if si is not None:
            for u in si.on_update or []:
                if u.ant_name:
                    own_count[u.ant_name] = own_count.get(u.ant_name, 0) + u.update_value

    # cumulative: after the k-th E-sem increment, the largest value of each
    # other semaphore that engine E has (transitively) waited on so far.
    def prefix(name):
        return name.split("_")[0]

    tick = {}  # sem prefix -> increments so far
    cur_max = {}  # engine prefix -> {other sem prefix -> max waited value}
    observed = {}  # (engine prefix, other prefix) -> [(tick, maxval)]
    for inst in insts:
        si = inst.sync_info
        if si is None:
            continue
        eng = str(inst.engine).split(".")[-1]
        cm = cur_max.setdefault(eng, {})
        for w in si.on_wait or []:
            if w.ant_name:
                p = prefix(w.ant_name)
                if p != eng:
                    cm[p] = max(cm.get(p, 0), w.wait_value)
        for u in si.on_update or []:
            if u.ant_name:
                p = prefix(u.ant_name)
                tick[p] = tick.get(p, 0) + u.update_value
                if p == eng:
                    for q, v in cm.items():
                        observed.setdefault((p, q), []).append((tick[p], v))

    def implied(via_prefix, via_val, other_name, other_val):
        """True when "via-sem >= via_val" transitively implies
        "other-sem >= other_val": the via engine had waited on other-sem
        >= other_val by the time it made its via_val-th increment."""
        p = prefix(other_name)
        best = 0
        for k, v in observed.get((via_prefix, p), []):
            if k <= via_val:
                best = max(best, v)
        return best >= other_val

    # 2. split multi-wait drains
    for blk in f.blocks:
        new_list = []
        for inst in blk.instructions:
            si = inst.sync_info
            if (
                type(inst).__name__ == "InstDrain"
                and si is not None
                and si.on_wait
                and len(si.on_wait) > 1
            ):
                waits = list(si.on_wait)
                for k, w in enumerate(waits[:-1]):
                    clone = _copy.copy(inst)
                    clone.name = f"{inst.name}-w{k}"
                    clone.sync_info = _mybir.SyncInfo(on_wait=[w], on_update=[])
                    new_list.append(clone)
                inst.sync_info = _mybir.SyncInfo(
                    on_wait=[waits[-1]], on_update=list(si.on_update or [])
                )
            new_list.append(inst)
        blk.instructions[:] = new_list

    # 3. transitivity strip: for each wait, check whether one of the OTHER
    # waits on the instruction already implies it; drop implied waits.
    for inst in insts:
        si = inst.sync_info
        if si is None or not si.on_wait or len(si.on_wait) <= 1:
            continue
        waits = list(si.on_wait)
        keep = []
        for i, w in enumerate(waits):
            redundant = any(
                implied(prefix(v.ant_name), v.wait_value, w.ant_name, w.wait_value)
                for j, v in enumerate(waits)
                if j != i and (v in keep or j > i)
            )
            if not redundant:
                keep.append(w)
        if len(keep) < len(waits):
            inst.sync_info = _mybir.SyncInfo(
                on_wait=keep, on_update=list(si.on_update or [])
            )

    # 4. fallback: split residual multi-wait compute instructions by hoisting
    # all but one wait onto single-wait drains inserted just before them on
    # the same engine (a satisfied drain retires in a few ns).
    n_split = 0
    for blk in f.blocks:
        new_list = []
        for inst in blk.instructions:
            si = inst.sync_info
            if (
                si is not None
                and si.on_wait
                and len(si.on_wait) > 1
                and type(inst).__name__
                not in ("InstDrain", "InstEventSemaphore", "InstISA", "InstCall")
            ):
                waits = list(si.on_wait)
                for k, w in enumerate(waits[:-1]):
                    d = _mybir.InstDrain(
                        name=f"{inst.name}-sw{k}",
                        ins=[],
                        outs=[],
                        bass_is_fusable=False,
                    )
                    d.engine = inst.engine
                    d.sync_info = _mybir.SyncInfo(on_wait=[w], on_update=[])
                    new_list.append(d)
                    n_split += 1
                inst.sync_info = _mybir.SyncInfo(
                    on_wait=[waits[-1]], on_update=list(si.on_update or [])
                )
            new_list.append(inst)
        blk.instructions[:] = new_list


def _make_const_inputs(transitions, start_transitions, stop_transitions):
    transitions = np.asarray(transitions, dtype=np.float32)
    start = np.asarray(start_transitions, dtype=np.float32)
    stop = np.asarray(stop_transitions, dtype=np.float32)
    consts = np.zeros((B_LOC, NCONST), dtype=np.float32)
    o = 0
    consts[:, o : o + T * T] = transitions.T.reshape(1, T * T)  # [c*T+q] = trans[q,c]
    o += T * T
    consts[:, o : o + T * T] = transitions.reshape(1, T * T)  # [m*T+r] = trans[m,r]
    o += T * T
    consts[:, o : o + T] = np.tile(transitions.T, (4, 1))  # transmmF[p,f]=trans[f,p%32]
    o += T
    consts[:, o : o + T] = np.tile(transitions, (4, 1))  # transmmB[p,f]=trans[p%32,f]
    o += T
    consts[:, o : o + T] = start[None, :]
    o += T
    consts[:, o : o + T] = stop[None, :]
    o += T
    # idxW[p, k] = p%32 if p//32==k else 0 (block-selective iota for the
    # path-index PE matmul); rest of the slot unused.
    p = np.arange(4 * T)
    consts[:, o : o + 4] = ((p[:, None] // T) == np.arange(4)[None, :]) * (
        p[:, None] % T
    ).astype(np.float32)
    o += T
    consts[:, o : o + 2 * T] = np.tile(np.arange(T, dtype=np.float32), 2)[None, :]
    return {"consts": consts}


class Runner:
    """Compile once, keep inputs device-resident, execute repeatedly."""

    def __init__(self, nc, n_cores=N_CORES):
        import jax
        from jax.sharding import Mesh, PartitionSpec, NamedSharding
        from jax.experimental.shard_map import shard_map
        from concourse import bass2jax

        self.jax = jax
        bass2jax.install_neuronx_cc_hook()

        partition_name = (
            nc.partition_id_tensor.name if nc.partition_id_tensor else None
        )
        in_names, out_names, out_avals, zero_outs = [], [], [], []
        for alloc in nc.m.functions[0].allocations:
            if not isinstance(alloc, mybir.MemoryLocationSet):
                continue
            name = alloc.memorylocations[0].name
            if alloc.kind == "ExternalInput":
                if name != partition_name:
                    in_names.append(name)
            elif alloc.kind == "ExternalOutput":
                out_names.append(name)
                shape = tuple(alloc.tensor_shape)
                dtype = mybir.dt.np(alloc.dtype)
                out_avals.append(jax.core.ShapedArray(shape, dtype))
                zero_outs.append(np.zeros((n_cores * shape[0], *shape[1:]), dtype))
        n_params = len(in_names)
        all_names = in_names + out_names
        if partition_name is not None:
            all_names = all_names + [partition_name]

        def _body(*args):
            operands = list(args)
            if partition_name is not None:
                operands.append(bass2jax.partition_id_tensor())
            outs = bass2jax._bass_exec_p.bind(
                *operands,
                out_avals=tuple(out_avals),
                in_names=tuple(all_names),
                out_names=tuple(out_names),
                lowering_input_output_aliases=(),
                sim_require_finite=True,
                sim_require_nnan=True,
                nc=nc,
            )
            return tuple(outs)

        self._body = _body
        devices = jax.devices()[:n_cores]
        assert len(devices) == n_cores
        self.mesh = Mesh(np.asarray(devices), ("core",))
        in_specs = (PartitionSpec("core"),) * (n_params + len(out_names))
        out_specs = (PartitionSpec("core"),) * len(out_names)
        self.sharded = jax.jit(
            shard_map(
                _body,
                mesh=self.mesh,
                in_specs=in_specs,
                out_specs=out_specs,
                check_rep=False,
            ),
            donate_argnums=tuple(range(n_params, n_params + len(out_names))),
            keep_unused=True,
        )
        self.sharding = NamedSharding(self.mesh, PartitionSpec("core"))
        self.in_names = in_names
        self.out_names = out_names
        self.out_avals = out_avals
        self.zero_outs = zero_outs
        self.n_cores = n_cores
        self.dev_in = None

    def set_inputs(self, in_maps):
        concat = [
            np.concatenate([np.asarray(m[name]) for m in in_maps], axis=0)
            for name in self.in_names
        ]
        self.dev_in = [self.jax.device_put(a, self.sharding) for a in concat]

    def execute(self):
        outs = self.sharded(*self.dev_in, *[z.copy() for z in self.zero_outs])
        outs = self.jax.block_until_ready(outs)
        return {
            name: np.asarray(outs[i]).reshape(
                self.n_cores, *self.out_avals[i].shape
            )
            for i, name in enumerate(self.out_names)
        }

    def make_chained(self, n_chain):
        """Callable dispatching the NEFF n_chain times, each execution's
        outputs threaded in as the next one's output-seed operands (data
        dependency serializes them on device); blocks once at the end.
        Wall-time slope over n_chain isolates on-device execution time from
        per-call host/RPC overhead."""
        import jax
        from jax.experimental.shard_map import shard_map
        from jax.sharding import PartitionSpec

        n_params = len(self.in_names)
        in_specs = (PartitionSpec("core"),) * (n_params + len(self.out_names))
        out_specs = (PartitionSpec("core"),) * len(self.out_names)
        fn = jax.jit(
            shard_map(
                self._body,
                mesh=self.mesh,
                in_specs=in_specs,
                out_specs=out_specs,
                check_rep=False,
            ),
            keep_unused=True,
        )
        dev_zeros = [self.jax.device_put(z, self.sharding) for z in self.zero_outs]

        def run():
            outs = tuple(dev_zeros)
            for _ in range(n_chain):
                outs = fn(*self.dev_in, *outs)
            return self.jax.block_until_ready(outs)

        return run


_RUNNER_CACHE = {}


def _get_runner(S, kind="main"):
    key = (S, kind)
    if key not in _RUNNER_CACHE:
        nc = build_nc(S) if kind == "main" else build_noop_nc(S)
        _RUNNER_CACHE[key] = Runner(nc)
    return _RUNNER_CACHE[key]


def build_noop_nc(S):
    """Same I/O signature, near-zero device work — for launch-overhead calibration."""
    nc = bass.Bass()
    nc.declare_dram_parameter("feats", [B_LOC, S, T], F32, isOutput=False)
    consts_d = nc.declare_dram_parameter("consts", [B_LOC, NCONST], F32, isOutput=False)
    path_d = nc.declare_dram_parameter("path", [32, S * 4], F32, isOutput=True)
    with TileContext(nc) as tc:
        with tc.tile_pool(name="w", bufs=1) as pool:
            t = pool.tile([32, T], F32)
            nc.sync.dma_start(out=t[:], in_=consts_d[0:32, 0:T])
            ti = pool.tile([32, T], F32)
            nc.vector.tensor_copy(ti[:], t[:])
            nc.sync.dma_start(out=path_d[:, 0:T], in_=ti[:])
    _strip_redundant_pe_waits(nc)
    return nc


def _in_maps_for(feats, transitions, start_transitions, stop_transitions, n_cores):
    consts = _make_const_inputs(transitions, start_transitions, stop_transitions)
    in_maps = []
    for c in range(n_cores):
        m = dict(consts)
        m["feats"] = np.ascontiguousarray(feats[c * B_LOC : (c + 1) * B_LOC])
        in_maps.append(m)
    return in_maps


def run_on_cores(feats, transitions, start_transitions, stop_transitions, trace=False):
    feats = np.asarray(feats, dtype=np.float32)
    B, S, T_ = feats.shape
    assert T_ == T and B % B_LOC == 0
    n_cores = B // B_LOC
    runner = _get_runner(S)
    runner.set_inputs(
        _in_maps_for(feats, transitions, start_transitions, stop_transitions, n_cores)
    )
    out = runner.execute()["path"]
    # device layout: per core [32, S, 4] with path[32*k + b', s] = out[b', s, k]
    out = out.reshape(n_cores, 32, S, 4).transpose(0, 3, 1, 2)
    return np.ascontiguousarray(out).reshape(B, S).astype(np.int32), None


def kernel(feats, tags, transitions, start_transitions, stop_transitions):
    out, _ = run_on_cores(feats, transitions, start_transitions, stop_transitions)
    return out

